# revision 3
# baseline (speedup 1.0000x reference)
# Trainium2 Bass kernel for nn_EssentialToPose.
#
# Pipeline:
#   host (CPU, tiny O(B) work): batched 3x3 SVD of essential_raw + the
#     per-sample pose candidates r1, r2, t.  This must be computed with the
#     same LAPACK the reference uses: ~44% of the samples land on exact
#     chirality-vote ties, where the reference's strict `>` tie-break selects
#     r2/-t, and *which* matrix is "r2" depends on the SVD's per-column sign
#     convention.  Reproducing LAPACK sgesdd's signs on-device is not
#     feasible; everything downstream of the SVD (99.99% of FLOPs / bytes:
#     the B x N x 4-hypothesis triangulation + chirality voting + pose
#     selection + output assembly) runs on the 8 NeuronCores.
#   device (8 cores, batch-parallel, 256 samples/core): for every point n and
#     both rotations, evaluate the two division-free depth signs
#        z1 = alpha - beta*d2z + g*tz        (sign of depth in cam 1 * 2g)
#        z2 = alpha*cd1 - beta - g*ct        (sign of depth in cam 2 * 2g)
#     (g = |p1|^2|p2|^2 - (p1.R p2)^2 >= 0 by Cauchy-Schwarz, so multiplying
#     the reference's divisions through by 2g preserves the strict sign
#     tests), count sign patterns, vote, select pose, assemble [B,4,4].
#
#   Algebraic reductions used on-device:
#     q = R^T t = u[:,:,2]  for BOTH rotations (t = v e3, W^T e3 = W e3 = e3)
#     ct = t . R[:,2] = u22 for both rotations
#     r2 = 2 v2 u2^T - r1   => dot12/d2z/cd1 for r2 are 1 fused op each.

import numpy as np

B, N, NCORES = 2048, 2048, 8
BC = B // NCORES          # samples per core
P = 128                   # SBUF partitions
NTILE = BC // P           # sample tiles per core
NCHUNK = 2                # chunks over the N points
NC = N // NCHUNK

_prog_cache = {}


def _host_prep(essential_raw, c2d2d, k_matrix):
    """CPU-side per-sample prep, mirroring reference.py ops bit-for-bit."""
    import jax
    cpu = jax.devices("cpu")[0]
    import jax.numpy as jnp

    with jax.default_device(cpu):
        E = jnp.asarray(np.ascontiguousarray(np.asarray(essential_raw)))
        U, S, Vh = jnp.linalg.svd(E, full_matrices=False)
        V = jnp.swapaxes(Vh, 1, 2)
        u = U * jnp.sign(jnp.linalg.det(U))[:, None, None]
        v = V * jnp.sign(jnp.linalg.det(V))[:, None, None]
        last_svs = S[:, -1]

        W = jnp.array([[0.0, -1.0, 0.0], [1.0, 0.0, 0.0], [0.0, 0.0, 1.0]],
                      dtype=jnp.float32)
        ut = jnp.swapaxes(u, 1, 2)
        r1 = v @ W @ ut
        r2 = v @ W.T @ ut
        t = v[:, :, 2]
        # reference normalizes after sign selection; (-t)/||t|| == -(t/||t||)
        # exactly in IEEE, so the device just flips the sign of t_unit.
        t_unit = t / jnp.sqrt((t ** 2).sum(axis=-1, keepdims=True))

        k_inv = jnp.linalg.inv(jnp.asarray(np.asarray(k_matrix)))

        r1 = np.asarray(r1); r2 = np.asarray(r2)
        t = np.asarray(t); un = np.asarray(u)
        t_unit = np.asarray(t_unit)
        last_svs = np.asarray(last_svs)
        k_inv = np.asarray(k_inv)

    q = un[:, :, 2]                      # R^T t for both rotations
    coef = np.zeros((B, 32), np.float32)
    coef[:, 0:9] = r1.reshape(B, 9)
    coef[:, 9:18] = r2.reshape(B, 9)
    coef[:, 18:21] = t
    coef[:, 21:24] = q
    coef[:, 24] = np.float32(2.0) * t[:, 2]     # 2*tz   (d2z_r2 fold)
    coef[:, 25] = np.float32(2.0) * q[:, 2]     # 2*ct   (cd1_r2 fold)
    coef[:, 26] = -q[:, 2]                      # -ct    (z2 fused scale)
    coef[:, 27:30] = t_unit

    kc = (float(k_inv[0, 0]), float(k_inv[0, 2]),
          float(k_inv[1, 1]), float(k_inv[1, 2]))
    return coef, last_svs, kc


def _emit(tc, pts, coef, tf, kc):
    from contextlib import ExitStack
    import concourse.bass as bass  # noqa: F401
    from concourse import mybir

    nc = tc.nc
    f32 = mybir.dt.float32
    A = mybir.AluOpType
    AF = mybir.ActivationFunctionType
    ka0, kb0, ka1, kb1 = kc

    with ExitStack() as ctx:
        io = ctx.enter_context(tc.tile_pool(name="io", bufs=2))
        cpool = ctx.enter_context(tc.tile_pool(name="cpool", bufs=2))
        fp = ctx.enter_context(tc.tile_pool(name="fp", bufs=1))
        sp = ctx.enter_context(tc.tile_pool(name="sp", bufs=2))

        for it in range(NTILE):
            s0 = it * P
            cf = cpool.tile([P, 32], f32, tag="cf", name="cf")
            nc.sync.dma_start(out=cf, in_=coef[s0:s0 + P, :])

            def C(i):
                return cf[:, i:i + 1]

            # per-chunk partial counts: [r1t | r1m | r2t | r2m] x NCHUNK cols
            cnts = sp.tile([P, 4 * NCHUNK], f32, tag="cnts", name="cnts")

            for k in range(NCHUNK):
                def T(tag):
                    return fp.tile([P, NC], f32, tag=tag, name=tag)

                blk = io.tile([P, NC, 4], f32, tag="blk", name="blk")
                nc.sync.dma_start(out=blk, in_=pts[s0:s0 + P, k * NC:(k + 1) * NC, :])

                # normalized ray xy components (ACT, strided read)
                p1x = T("p1x"); p1y = T("p1y"); p2x = T("p2x"); p2y = T("p2y")
                nc.scalar.activation(p1x, blk[:, :, 0], AF.Copy, bias=kb0, scale=ka0)
                nc.scalar.activation(p1y, blk[:, :, 1], AF.Copy, bias=kb1, scale=ka1)
                nc.scalar.activation(p2x, blk[:, :, 2], AF.Copy, bias=kb0, scale=ka0)
                nc.scalar.activation(p2y, blk[:, :, 3], AF.Copy, bias=kb1, scale=ka1)

                # |p1|^2, |p2|^2, product
                sq1 = T("sq1"); sq2 = T("sq2"); sq3 = T("sq3"); sq4 = T("sq4")
                nc.scalar.square(sq1, p1x)
                nc.scalar.square(sq2, p1y)
                nc.scalar.square(sq3, p2x)
                nc.scalar.square(sq4, p2y)
                d11 = T("d11"); d22 = T("d22"); dd = T("dd")
                nc.vector.scalar_tensor_tensor(d11, sq2, 1.0, sq1, A.add, A.add)
                nc.vector.scalar_tensor_tensor(d22, sq4, 1.0, sq3, A.add, A.add)
                nc.vector.tensor_mul(dd, d11, d22)

                # outer products
                oa = T("oa"); ob = T("ob"); oc = T("oc"); od = T("od")
                nc.vector.tensor_mul(oa, p1x, p2x)
                nc.vector.tensor_mul(ob, p1x, p2y)
                nc.vector.tensor_mul(oc, p1y, p2x)
                nc.vector.tensor_mul(od, p1y, p2y)

                # t.p1 and q.p2
                tp1 = T("tp1"); qp2 = T("qp2")
                nc.scalar.activation(tp1, p1x, AF.Identity, bias=C(20), scale=C(18))
                nc.vector.scalar_tensor_tensor(tp1, p1y, C(19), tp1, A.mult, A.add)
                nc.scalar.activation(qp2, p2x, AF.Identity, bias=C(23), scale=C(21))
                nc.vector.scalar_tensor_tensor(qp2, p2y, C(22), qp2, A.mult, A.add)

                X = T("X"); Y = T("Y")
                nc.vector.tensor_mul(X, tp1, d22)
                nc.vector.tensor_mul(Y, qp2, d11)

                d12 = T("d12"); d2z = T("d2z"); cd1 = T("cd1")
                gs = T("gs"); g = T("g")
                u1 = T("u1"); u2 = T("u2"); al = T("al"); be = T("be")
                z1 = T("z1"); z2 = T("z2"); mn = T("mn"); mx = T("mx")
                ind = T("ind")

                for r, (cb, ct_t, cm_t) in enumerate((("r1", 0, 1), ("r2", 2, 3))[:2]):
                    if r == 0:
                        # dot12 = p1^T r1 p2 (9-term chain), d2z, cd1
                        nc.scalar.activation(d12, oa, AF.Identity, bias=C(8), scale=C(0))
                        for feat, ci in ((ob, 1), (p1x, 2), (oc, 3), (od, 4),
                                         (p1y, 5), (p2x, 6), (p2y, 7)):
                            nc.vector.scalar_tensor_tensor(d12, feat, C(ci), d12,
                                                           A.mult, A.add)
                        nc.scalar.activation(d2z, p2x, AF.Identity, bias=C(8), scale=C(6))
                        nc.vector.scalar_tensor_tensor(d2z, p2y, C(7), d2z, A.mult, A.add)
                        nc.scalar.activation(cd1, p1x, AF.Identity, bias=C(8), scale=C(2))
                        nc.vector.scalar_tensor_tensor(cd1, p1y, C(5), cd1, A.mult, A.add)
                    else:
                        # r2 = 2 v2 u2^T - r1 folds
                        tq = u1  # u1 is free here; reuse as scratch
                        nc.vector.tensor_mul(tq, tp1, qp2)
                        nc.vector.scalar_tensor_tensor(d12, tq, 2.0, d12,
                                                       A.mult, A.subtract)
                        nc.vector.scalar_tensor_tensor(d2z, qp2, C(24), d2z,
                                                       A.mult, A.subtract)
                        nc.vector.scalar_tensor_tensor(cd1, tp1, C(25), cd1,
                                                       A.mult, A.subtract)

                    nc.scalar.square(gs, d12)
                    nc.vector.tensor_sub(g, dd, gs)
                    nc.vector.tensor_mul(u1, qp2, d12)
                    nc.vector.tensor_sub(al, X, u1)
                    nc.vector.tensor_mul(u2, tp1, d12)
                    nc.vector.tensor_sub(be, Y, u2)
                    # z1 = al - be*d2z + g*tz
                    nc.vector.tensor_mul(u1, be, d2z)
                    nc.vector.tensor_sub(u2, al, u1)
                    nc.vector.scalar_tensor_tensor(z1, g, C(20), u2, A.mult, A.add)
                    # z2 = al*cd1 - be - g*ct
                    nc.vector.tensor_mul(u1, al, cd1)
                    nc.vector.tensor_sub(u2, u1, be)
                    nc.vector.scalar_tensor_tensor(z2, g, C(26), u2, A.mult, A.add)

                    nc.vector.tensor_tensor(mn, z1, z2, A.min)
                    nc.vector.tensor_tensor(mx, z1, z2, A.max)
                    nc.vector.tensor_scalar(ind, mn, 0.0, None, A.is_gt, A.add,
                                            accum_out=cnts[:, ct_t * NCHUNK + k:
                                                           ct_t * NCHUNK + k + 1])
                    nc.vector.tensor_scalar(ind, mx, 0.0, None, A.is_lt, A.add,
                                            accum_out=cnts[:, cm_t * NCHUNK + k:
                                                           cm_t * NCHUNK + k + 1])

            # ---- votes + pose selection + assembly ----
            sc = sp.tile([P, 4], f32, tag="sc", name="sc")
            for c in range(4):
                nc.vector.reduce_sum(sc[:, c:c + 1],
                                     cnts[:, c * NCHUNK:(c + 1) * NCHUNK],
                                     axis=mybir.AxisListType.X)
            vr1 = sp.tile([P, 1], f32, tag="vr1", name="vr1"); vr2 = sp.tile([P, 1], f32, tag="vr2", name="vr2")
            vt1 = sp.tile([P, 1], f32, tag="vt1", name="vt1"); vt2 = sp.tile([P, 1], f32, tag="vt2", name="vt2")
            nc.vector.tensor_add(vr1, sc[:, 0:1], sc[:, 1:2])
            nc.vector.tensor_add(vr2, sc[:, 2:3], sc[:, 3:4])
            nc.vector.tensor_add(vt1, sc[:, 0:1], sc[:, 2:3])
            nc.vector.tensor_add(vt2, sc[:, 1:2], sc[:, 3:4])
            mask_r = sp.tile([P, 1], f32, tag="mask_r", name="mask_r")
            sgn_t = sp.tile([P, 1], f32, tag="sgn_t", name="sgn_t")
            nc.vector.tensor_tensor(mask_r, vr1, vr2, A.is_gt)
            nc.vector.tensor_tensor(sgn_t, vt1, vt2, A.is_gt)
            nc.vector.tensor_scalar(sgn_t, sgn_t, 2.0, -1.0, A.mult, A.add)

            rdiff = sp.tile([P, 9], f32, tag="rdiff", name="rdiff")
            nc.vector.tensor_sub(rdiff, cf[:, 0:9], cf[:, 9:18])

            tft = sp.tile([P, 16], f32, tag="tft", name="tft")
            nc.vector.memset(tft[:, 12:16], 0.0)
            nc.vector.memset(tft[:, 15:16], 1.0)
            tft4 = tft.rearrange("p (i j) -> p i j", j=4)
            r_view = tft4[:, 0:3, 0:3]
            t_view = tft4[:, 0:3, 3]
            nc.vector.scalar_tensor_tensor(
                r_view, rdiff.rearrange("p (i j) -> p i j", j=3), mask_r,
                cf[:, 9:18].rearrange("p (i j) -> p i j", j=3), A.mult, A.add)
            nc.vector.tensor_scalar(t_view, cf[:, 27:30], sgn_t, None, A.mult)

            nc.sync.dma_start(out=tf[s0:s0 + P, :], in_=tft)


def _build_program(kc):
    key = kc
    if key in _prog_cache:
        return _prog_cache[key]
    import concourse.tile as tile
    from concourse import bacc, mybir

    nc = bacc.Bacc("TRN2", target_bir_lowering=False, debug=False,
                   num_devices=NCORES)
    pts = nc.dram_tensor("pts", [BC, N, 4], mybir.dt.float32,
                         kind="ExternalInput").ap()
    coef = nc.dram_tensor("coef", [BC, 32], mybir.dt.float32,
                          kind="ExternalInput").ap()
    tf = nc.dram_tensor("tf", [BC, 16], mybir.dt.float32,
                        kind="ExternalOutput").ap()
    with tile.TileContext(nc) as tc:
        _emit(tc, pts, coef, tf, kc)
    nc.compile()
    _prog_cache[key] = nc
    return nc


def kernel(essential_raw, c2d2d, k_matrix):
    from concourse import bass_utils

    essential_raw = np.asarray(essential_raw, np.float32)
    c2d2d = np.ascontiguousarray(np.asarray(c2d2d, np.float32))
    k_matrix = np.asarray(k_matrix, np.float32)

    coef, last_svs, kc = _host_prep(essential_raw, c2d2d, k_matrix)
    nc = _build_program(kc)

    in_maps = [
        {"pts": c2d2d[i * BC:(i + 1) * BC], "coef": coef[i * BC:(i + 1) * BC]}
        for i in range(NCORES)
    ]
    res = bass_utils.run_bass_kernel_spmd(nc, in_maps, core_ids=list(range(NCORES)))
    tf = np.concatenate([np.asarray(r["tf"]) for r in res.results], axis=0)
    return tf.reshape(B, 4, 4), last_svs
